# revision 9
# baseline (speedup 1.0000x reference)
"""AtomAttentionEncoder on 8 NeuronCores.

Sharding strategy (hardcoded, matches the hint): data-parallel over the
subset/block dim S. Each of the 8 cores owns 64 of the 512 query subsets
(2048 atoms / 512 tokens). The clipped sliding key window (128 wide,
reaching 48 atoms left / 48 right of a 32-atom query block) is handled by
giving every core a pre-clamped *halo* of input atoms instead of a runtime
exchange: because qact is updated NB=3 times and each update reads +-48
atoms, a +-4-subset compute halo (+-2 more subsets of read-only input)
makes the owned 64 subsets bit-correct with zero inter-core traffic.

Per core:
  - input atoms  j in [0, 2432)  <-> global clip(2048c - 192 + j, 0, N-1)
  - extended subsets ls in [0,72) <-> global s = 64c - 4 + ls
    query atoms of ls: j in [32ls+64, 32ls+96)
    key window of ls:  j in [32ls+16, 32ls+144)   (no local clipping needed;
    the host-side clip reproduces the reference's per-index clip exactly,
    since clip is monotone and windows are contiguous)
  - after 3 blocks the middle 64 subsets (ls in [4,68)) are exact; they are
    the shard's contribution to the output.
"""

import numpy as np

T, A = 4096, 4
N = T * A
QB, KB = 32, 128
S = N // QB
H, DK = 4, 32
NB = 3
CA, CP, CT = 128, 16, 384

M = 8                     # cores
S_OWN = S // M            # 64 owned subsets / core
HALO_S = 4                # compute halo (subsets) each side
S_EXT = S_OWN + 2 * HALO_S          # 72 extended subsets
ATOMS_LOC = (S_EXT + 4) * QB        # 2432 input atoms / core (2 more subsets each side)
ATOM0 = lambda c: 2048 * c - 192    # global index of local atom 0 (pre-clip)
Q_OFF = 64                          # local atom offset of first extended-subset query atom
OWN_Q_LO, OWN_Q_HI = 192, 2240      # local atom range owned (64 subsets)

_WNAMES = ['w_ref_pos', 'w_ref_mask', 'w_ref_element', 'w_ref_charge', 'w_ref_atom_name',
           'w_s2p_row', 'w_s2p_col', 'w_pair_offsets', 'w_pair_dist', 'w_pair_mask',
           'w_pair_mlp1', 'w_pair_mlp2', 'w_pair_mlp3', 'w_pair_ln', 'w_pair_logits',
           'aq_lnw', 'aq_gw', 'aq_gb', 'aq_sw', 'ak_lnw', 'ak_gw', 'ak_gb', 'ak_sw',
           'wq', 'bq', 'wk', 'wv', 'wg', 'azi_w', 'azi_cw', 'azi_cb',
           't_lnw', 't_gw', 't_gb', 't_sw', 'glu1', 'glu2',
           't_azi_w', 't_azi_cw', 't_azi_cb', 'w_project']

_compiled = None


def _build():
    import jax, jax.numpy as jnp

    devs = [d for d in jax.devices() if d.platform != 'cpu'][:M]
    if len(devs) < M:
        devs = jax.devices()[:M]
    assert len(devs) == M, f"need {M} devices, got {devs}"

    # local clipped key-window gather: [S_EXT, KB]
    KIDX_L = np.arange(S_EXT)[:, None] * QB + 16 + np.arange(KB)[None, :]

    bf16 = jnp.bfloat16
    f32 = jnp.float32

    def bmm(a, b):
        # bf16 inputs, f32 accumulate: TRN2 PE runs bf16 at 4x the fp32 rate
        return jnp.matmul(a.astype(bf16), b.astype(bf16),
                          preferred_element_type=f32)

    def _ln(x, w=None, eps=1e-5):
        mu = jnp.mean(x, -1, keepdims=True)
        var = jnp.var(x, -1, keepdims=True)
        xn = (x - mu) * jax.lax.rsqrt(var + eps)
        return xn * w if w is not None else xn

    def _ada_ln(x, cond, lnw, gw, gb, sw):
        c = _ln(cond, lnw)
        return jax.nn.sigmoid(bmm(c, gw) + gb) * _ln(x) + bmm(c, sw)

    def shard_fwd(positions, mask, element, charge, atom_name_chars, ref_space_uid,
                  w_ref_pos, w_ref_mask, w_ref_element, w_ref_charge, w_ref_atom_name,
                  w_s2p_row, w_s2p_col, w_pair_offsets, w_pair_dist, w_pair_mask,
                  w_pair_mlp1, w_pair_mlp2, w_pair_mlp3, w_pair_ln, w_pair_logits,
                  aq_lnw, aq_gw, aq_gb, aq_sw, ak_lnw, ak_gw, ak_gb, ak_sw,
                  wq, bq, wk, wv, wg, azi_w, azi_cw, azi_cb,
                  t_lnw, t_gw, t_gb, t_sw, glu1, glu2, t_azi_w, t_azi_cw, t_azi_cb,
                  w_project):
        dt = positions.dtype
        nl = ATOMS_LOC
        pos = positions.reshape(nl, 3)
        m = mask.reshape(nl, 1)
        # per_atom_cond over all local atoms
        el1h = jax.nn.one_hot(element.reshape(nl), 128, dtype=dt)
        nm1h = jax.nn.one_hot(atom_name_chars.reshape(nl, 4), 64, dtype=dt).reshape(nl, 256)
        chx = charge.reshape(nl, 1)
        ch = jnp.sign(chx) * jnp.log(jnp.abs(chx) + jnp.sqrt(chx * chx + 1.0))
        act = pos @ w_ref_pos + m @ w_ref_mask + bmm(el1h, w_ref_element) + ch @ w_ref_charge + bmm(nm1h, w_ref_atom_name)
        act = act * m                                     # [nl, CA]

        qatoms = jnp.arange(S_EXT * QB) + Q_OFF           # local query atoms of ext subsets
        qsc = act[qatoms].reshape(S_EXT, QB, CA)
        uid = ref_space_uid.reshape(nl)
        q_uid = uid[qatoms].reshape(S_EXT, QB)
        k_uid = uid[KIDX_L]
        q_pos = pos[qatoms].reshape(S_EXT, QB, 3)
        k_pos = pos[KIDX_L]
        valid = (q_uid[:, :, None] == k_uid[:, None, :]).astype(dt)[..., None]
        off = q_pos[:, :, None, :] - k_pos[:, None, :, :]
        pair = (off @ w_pair_offsets) * valid
        sq = jnp.sum(off * off, -1, keepdims=True)
        pair = pair + ((1.0 / (1.0 + sq)) @ w_pair_dist) * valid
        ksc = act[KIDX_L]
        pair = pair + bmm(jax.nn.relu(qsc), w_s2p_row)[:, :, None, :] + bmm(jax.nn.relu(ksc), w_s2p_col)[:, None, :, :]
        pair = pair + valid @ w_pair_mask
        t = bmm(jax.nn.relu(pair), w_pair_mlp1)
        t = bmm(jax.nn.relu(t), w_pair_mlp2)
        t = bmm(jax.nn.relu(t), w_pair_mlp3)
        pair = pair + t
        pl = bmm(_ln(pair, w_pair_ln), w_pair_logits).reshape(S_EXT, QB, KB, NB, H)
        pl = jnp.transpose(pl, (3, 0, 4, 1, 2))

        act_full = act                                    # atoms outside ext subsets stay at cond value
        qact = qsc
        scale = DK ** -0.5
        for i in range(NB):
            kact = act_full[KIDX_L]
            xq = _ada_ln(qact, qsc, aq_lnw[i], aq_gw[i], aq_gb[i], aq_sw[i])
            xk = _ada_ln(kact, ksc, ak_lnw[i], ak_gw[i], ak_gb[i], ak_sw[i])
            q = (bmm(xq, wq[i]) + bq[i]).reshape(S_EXT, QB, H, DK)
            k = bmm(xk, wk[i]).reshape(S_EXT, KB, H, DK)
            logits = jnp.einsum('sqhd,skhd->shqk', (q * scale).astype(bf16), k.astype(bf16),
                                preferred_element_type=f32) + pl[i]
            aw = jax.nn.softmax(logits, axis=-1)
            v = bmm(xk, wv[i]).reshape(S_EXT, KB, H, DK)
            o = jnp.einsum('shqk,skhd->sqhd', aw.astype(bf16), v.astype(bf16),
                           preferred_element_type=f32).reshape(S_EXT, QB, CA)
            o = o * jax.nn.sigmoid(bmm(xq, wg[i]))
            o = bmm(o, azi_w[i]) * jax.nn.sigmoid(bmm(qsc, azi_cw[i]) + azi_cb[i])
            qact = qact + o
            xt = _ada_ln(qact, qsc, t_lnw[i], t_gw[i], t_gb[i], t_sw[i])
            xt = jax.nn.silu(bmm(xt, glu1[i])) * bmm(xt, glu2[i])
            xt = bmm(xt, t_azi_w[i]) * jax.nn.sigmoid(bmm(qsc, t_azi_cw[i]) + t_azi_cb[i])
            qact = qact + xt
            act_full = act_full.at[Q_OFF:Q_OFF + S_EXT * QB].set(qact.reshape(S_EXT * QB, CA))

        # owned 64 subsets
        own = slice(HALO_S, HALO_S + S_OWN)
        q_own = qact[own].reshape(S_OWN * QB, CA)         # [2048, CA]
        tok = jax.nn.relu(bmm(q_own.reshape(T // M, A, CA), w_project))
        mm = mask.reshape(nl, 1)[OWN_Q_LO:OWN_Q_HI].reshape(T // M, A, 1)
        token_act = jnp.sum(tok * mm, -2) / jnp.maximum(jnp.sum(mm, -2), 1e-10)
        return token_act, qact[own], qsc[own], pair[own]

    fn = jax.pmap(shard_fwd, devices=devs)
    return fn


_wcache = {"fp": None, "dev": None}
_last_args = None


def _fingerprint(ws):
    h = 0
    for w in ws:
        h = hash((h, w.shape, w.dtype.str, float(w.ravel()[:64].sum()),
                  float(w.ravel()[-64:].sum()), float(abs(w).sum())))
    return h


def _prep_args(inputs):
    import jax
    gidx = np.empty((M, ATOMS_LOC), np.int64)
    for c in range(M):
        gidx[c] = np.clip(ATOM0(c) + np.arange(ATOMS_LOC), 0, N - 1)

    def shard_atoms(x):
        flat = np.ascontiguousarray(x).reshape(N, *x.shape[2:])
        return flat[gidx]

    args = [shard_atoms(np.asarray(inputs[nm]))
            for nm in ['positions', 'mask', 'element', 'charge', 'atom_name_chars', 'ref_space_uid']]

    ws = [np.asarray(inputs[nm]) for nm in _WNAMES]
    fp = _fingerprint(ws)
    if _wcache["fp"] != fp:
        devs = [d for d in jax.devices() if d.platform != 'cpu'][:M] or jax.devices()[:M]
        _wcache["dev"] = [jax.device_put_replicated(w, devs) for w in ws]
        _wcache["fp"] = fp
    return args + _wcache["dev"]


def kernel(**inputs):
    global _compiled, _last_args
    if _compiled is None:
        _compiled = _build()
    args = _prep_args(inputs)
    _last_args = args
    token_act, qact, qsc, pair = _compiled(*args)
    token_act = np.asarray(token_act).reshape(T, CT)
    qact = np.asarray(qact).reshape(S, QB, CA)
    qsc = np.asarray(qsc).reshape(S, QB, CA)
    pair = np.asarray(pair).reshape(S, QB, KB, CP)
    return token_act, qact, qsc, pair


def device_exec_time(n=5):
    """Time the on-device SPMD execution alone (outputs left on device)."""
    import jax, time
    assert _last_args is not None
    best = float('inf')
    for _ in range(n):
        t0 = time.time()
        outs = _compiled(*_last_args)
        jax.tree.map(lambda x: x.block_until_ready(), outs)
        best = min(best, time.time() - t0)
    return best


# revision 17
# speedup vs baseline: 1.8081x; 1.8081x over previous
"""AtomAttentionEncoder on 8 NeuronCores.

Sharding strategy (hardcoded, matches the hint): data-parallel over the
subset/block dim S. Each of the 8 cores owns 64 of the 512 query subsets
(2048 atoms / 512 tokens). The clipped sliding key window (128 wide,
reaching 48 atoms left / 48 right of a 32-atom query block) is handled by
giving every core a pre-clamped *halo* of input atoms instead of a runtime
exchange: because qact is updated NB=3 times and each update reads +-48
atoms, a +-4-subset compute halo (+-2 more subsets of read-only input)
makes the owned 64 subsets bit-correct with zero inter-core traffic.

Per core:
  - input atoms  j in [0, 2432)  <-> global clip(2048c - 192 + j, 0, N-1)
  - extended subsets ls in [0,72) <-> global s = 64c - 4 + ls
    query atoms of ls: j in [32ls+64, 32ls+96)
    key window of ls:  j in [32ls+16, 32ls+144)   (no local clipping needed;
    the host-side clip reproduces the reference's per-index clip exactly,
    since clip is monotone and windows are contiguous)
  - after 3 blocks the middle 64 subsets (ls in [4,68)) are exact; they are
    the shard's contribution to the output.
"""

import numpy as np

T, A = 4096, 4
N = T * A
QB, KB = 32, 128
S = N // QB
H, DK = 4, 32
NB = 3
CA, CP, CT = 128, 16, 384

M = 8                     # cores
S_OWN = S // M            # 64 owned subsets / core
HALO_S = 4                # compute halo (subsets) each side
S_EXT = S_OWN + 2 * HALO_S          # 72 extended subsets
ATOMS_LOC = (S_EXT + 4) * QB        # 2432 input atoms / core (2 more subsets each side)
ATOM0 = lambda c: 2048 * c - 192    # global index of local atom 0 (pre-clip)
Q_OFF = 64                          # local atom offset of first extended-subset query atom
OWN_Q_LO, OWN_Q_HI = 192, 2240      # local atom range owned (64 subsets)

_WNAMES = ['w_ref_pos', 'w_ref_mask', 'w_ref_element', 'w_ref_charge', 'w_ref_atom_name',
           'w_s2p_row', 'w_s2p_col', 'w_pair_offsets', 'w_pair_dist', 'w_pair_mask',
           'w_pair_mlp1', 'w_pair_mlp2', 'w_pair_mlp3', 'w_pair_ln', 'w_pair_logits',
           'aq_lnw', 'aq_gw', 'aq_gb', 'aq_sw', 'ak_lnw', 'ak_gw', 'ak_gb', 'ak_sw',
           'wq', 'bq', 'wk', 'wv', 'wg', 'azi_w', 'azi_cw', 'azi_cb',
           't_lnw', 't_gw', 't_gb', 't_sw', 'glu1', 'glu2',
           't_azi_w', 't_azi_cw', 't_azi_cb', 'w_project']

_compiled = None


def _devices():
    import jax
    return [d for d in jax.devices() if d.platform != 'cpu'][:M]


def _build():
    import jax, jax.numpy as jnp

    devs = _devices()

    # Local clipped key-window "gather": window(ls) = atoms[32ls+16 : 32ls+144).
    # Expressed as 4 shifted non-overlapping strided views + concat instead of a
    # real gather — XLA-neuron lowers gathers ~2.6x slower than slices.
    NW = S_EXT + 3          # 75 window chunks of 32

    def win_slice(x):       # [ATOMS_LOC, ...] -> [S_EXT, KB, ...]
        import jax.numpy as jnp
        xc = x[16:16 + NW * QB].reshape((NW, QB) + x.shape[1:])
        return jnp.concatenate([xc[j:j + S_EXT] for j in range(4)], axis=1)

    bf16 = jnp.bfloat16
    f32 = jnp.float32

    def bmm(a, b):
        # bf16 inputs, f32 accumulate: TRN2 PE runs bf16 at 4x the fp32 rate
        return jnp.matmul(a.astype(bf16), b.astype(bf16),
                          preferred_element_type=f32)

    def _ln(x, w=None, eps=1e-5):
        mu = jnp.mean(x, -1, keepdims=True)
        var = jnp.var(x, -1, keepdims=True)
        xn = (x - mu) * jax.lax.rsqrt(var + eps)
        return xn * w if w is not None else xn

    def _ada_ln(x, cond, lnw, gw, gb, sw):
        c = _ln(cond, lnw)
        return jax.nn.sigmoid(bmm(c, gw) + gb) * _ln(x) + bmm(c, sw)

    def shard_fwd(positions, mask, element, charge, atom_name_chars, ref_space_uid,
                  w_ref_pos, w_ref_mask, w_ref_element, w_ref_charge, w_ref_atom_name,
                  w_s2p_row, w_s2p_col, w_pair_offsets, w_pair_dist, w_pair_mask,
                  w_pair_mlp1, w_pair_mlp2, w_pair_mlp3, w_pair_ln, w_pair_logits,
                  aq_lnw, aq_gw, aq_gb, aq_sw, ak_lnw, ak_gw, ak_gb, ak_sw,
                  wq, bq, wk, wv, wg, azi_w, azi_cw, azi_cb,
                  t_lnw, t_gw, t_gb, t_sw, glu1, glu2, t_azi_w, t_azi_cw, t_azi_cb,
                  w_project):
        dt = positions.dtype
        nl = ATOMS_LOC
        pos = positions.reshape(nl, 3)
        m = mask.reshape(nl, 1)
        # per_atom_cond over all local atoms
        el1h = jax.nn.one_hot(element.reshape(nl), 128, dtype=dt)
        nm1h = jax.nn.one_hot(atom_name_chars.reshape(nl, 4), 64, dtype=dt).reshape(nl, 256)
        chx = charge.reshape(nl, 1)
        ch = jnp.sign(chx) * jnp.log(jnp.abs(chx) + jnp.sqrt(chx * chx + 1.0))
        act = pos @ w_ref_pos + m @ w_ref_mask + bmm(el1h, w_ref_element) + ch @ w_ref_charge + bmm(nm1h, w_ref_atom_name)
        act = act * m                                     # [nl, CA]

        qatoms = jnp.arange(S_EXT * QB) + Q_OFF           # local query atoms of ext subsets
        qsc = act[qatoms].reshape(S_EXT, QB, CA)
        uid = ref_space_uid.reshape(nl)
        q_uid = uid[qatoms].reshape(S_EXT, QB)
        k_uid = win_slice(uid)
        q_pos = pos[qatoms].reshape(S_EXT, QB, 3)
        k_pos = win_slice(pos)
        valid = (q_uid[:, :, None] == k_uid[:, None, :]).astype(dt)[..., None]
        off = q_pos[:, :, None, :] - k_pos[:, None, :, :]
        pair = (off @ w_pair_offsets) * valid
        sq = jnp.sum(off * off, -1, keepdims=True)
        pair = pair + ((1.0 / (1.0 + sq)) @ w_pair_dist) * valid
        ksc = win_slice(act)
        pair = pair + bmm(jax.nn.relu(qsc), w_s2p_row)[:, :, None, :] + bmm(jax.nn.relu(ksc), w_s2p_col)[:, None, :, :]
        pair = pair + valid @ w_pair_mask
        t = bmm(jax.nn.relu(pair), w_pair_mlp1)
        t = bmm(jax.nn.relu(t), w_pair_mlp2)
        t = bmm(jax.nn.relu(t), w_pair_mlp3)
        pair = pair + t
        pl = bmm(_ln(pair, w_pair_ln), w_pair_logits).reshape(S_EXT, QB, KB, NB, H)
        pl = jnp.transpose(pl, (3, 0, 4, 1, 2))

        act_full = act                                    # atoms outside ext subsets stay at cond value
        qact = qsc
        scale = DK ** -0.5
        for i in range(NB):
            kact = win_slice(act_full)
            xq = _ada_ln(qact, qsc, aq_lnw[i], aq_gw[i], aq_gb[i], aq_sw[i])
            xk = _ada_ln(kact, ksc, ak_lnw[i], ak_gw[i], ak_gb[i], ak_sw[i])
            q = (bmm(xq, wq[i]) + bq[i]).reshape(S_EXT, QB, H, DK)
            k = bmm(xk, wk[i]).reshape(S_EXT, KB, H, DK)
            logits = jnp.einsum('sqhd,skhd->shqk', (q * scale).astype(bf16), k.astype(bf16),
                                preferred_element_type=f32) + pl[i]
            aw = jax.nn.softmax(logits, axis=-1)
            v = bmm(xk, wv[i]).reshape(S_EXT, KB, H, DK)
            o = jnp.einsum('shqk,skhd->sqhd', aw.astype(bf16), v.astype(bf16),
                           preferred_element_type=f32).reshape(S_EXT, QB, CA)
            o = o * jax.nn.sigmoid(bmm(xq, wg[i]))
            o = bmm(o, azi_w[i]) * jax.nn.sigmoid(bmm(qsc, azi_cw[i]) + azi_cb[i])
            qact = qact + o
            xt = _ada_ln(qact, qsc, t_lnw[i], t_gw[i], t_gb[i], t_sw[i])
            xt = jax.nn.silu(bmm(xt, glu1[i])) * bmm(xt, glu2[i])
            xt = bmm(xt, t_azi_w[i]) * jax.nn.sigmoid(bmm(qsc, t_azi_cw[i]) + t_azi_cb[i])
            qact = qact + xt
            act_full = act_full.at[Q_OFF:Q_OFF + S_EXT * QB].set(qact.reshape(S_EXT * QB, CA))

        # owned 64 subsets
        own = slice(HALO_S, HALO_S + S_OWN)
        q_own = qact[own].reshape(S_OWN * QB, CA)         # [2048, CA]
        tok = jax.nn.relu(bmm(q_own.reshape(T // M, A, CA), w_project))
        mm = mask.reshape(nl, 1)[OWN_Q_LO:OWN_Q_HI].reshape(T // M, A, 1)
        token_act = jnp.sum(tok * mm, -2) / jnp.maximum(jnp.sum(mm, -2), 1e-10)
        return token_act, qact[own], qsc[own], pair[own]

    if len(devs) == M:
        fn = jax.pmap(shard_fwd, devices=devs)
    else:  # no 8-core accelerator visible: correctness fallback (vmap on default backend)
        fn = jax.jit(jax.vmap(shard_fwd))
    return fn


_wcache = {"fp": None, "dev": None}
_last_args = None


def _fingerprint(ws):
    h = 0
    for w in ws:
        h = hash((h, w.shape, w.dtype.str, float(w.ravel()[:64].sum()),
                  float(w.ravel()[-64:].sum()), float(abs(w).sum())))
    return h


def _prep_args(inputs):
    import jax
    gidx = np.empty((M, ATOMS_LOC), np.int64)
    for c in range(M):
        gidx[c] = np.clip(ATOM0(c) + np.arange(ATOMS_LOC), 0, N - 1)

    def shard_atoms(x):
        flat = np.ascontiguousarray(x).reshape(N, *x.shape[2:])
        return flat[gidx]

    args = [shard_atoms(np.asarray(inputs[nm]))
            for nm in ['positions', 'mask', 'element', 'charge', 'atom_name_chars', 'ref_space_uid']]

    ws = [np.asarray(inputs[nm]) for nm in _WNAMES]
    fp = _fingerprint(ws)
    if _wcache["fp"] != fp:
        devs = _devices()
        if len(devs) == M:
            _wcache["dev"] = [jax.device_put_replicated(w, devs) for w in ws]
        else:
            _wcache["dev"] = [np.broadcast_to(w, (M,) + w.shape) for w in ws]
        _wcache["fp"] = fp
    return args + _wcache["dev"]


def kernel(**inputs):
    global _compiled, _last_args
    if _compiled is None:
        _compiled = _build()
    args = _prep_args(inputs)
    _last_args = args
    token_act, qact, qsc, pair = _compiled(*args)
    token_act = np.asarray(token_act).reshape(T, CT)
    qact = np.asarray(qact).reshape(S, QB, CA)
    qsc = np.asarray(qsc).reshape(S, QB, CA)
    pair = np.asarray(pair).reshape(S, QB, KB, CP)
    return token_act, qact, qsc, pair


def device_exec_time(n=5):
    """Time the on-device SPMD execution alone (outputs left on device)."""
    import jax, time
    assert _last_args is not None
    best = float('inf')
    for _ in range(n):
        t0 = time.time()
        outs = _compiled(*_last_args)
        jax.tree.map(lambda x: x.block_until_ready(), outs)
        best = min(best, time.time() - t0)
    return best
